# Initial kernel scaffold
#
"""Causal attention kernel for Trainium2 (Bass/Tile), data-parallel over batch.

Problem (hardcoded): x[64,512,1024] f32, Wq/Wk/Wv[1024,256], bq/bk/bv[256].
  q = x@Wq+bq ; k = x@Wk+bk ; v = x@Wv+bv
  out = softmax(causal(q k^T / sqrt(256))) @ v           -> [64,512,256]

Sharding: 8 NeuronCores, 8 batches per core (pure data parallel, weights
replicated, no collectives). Each core runs the same program on its shard.

Per-core pipeline (per batch):
  1. DMA x[b] -> SBUF [128,4,1024]; PE-transpose 128x128 tiles -> xT [128,8,512]
  2. qT/kT = W.T @ x.T via fp32r matmuls (d on partitions), bias folded into
     the PSUM->SBUF copy; q pre-scaled by 1/sqrt(d).
     v computed in natural layout [tk,128-chunks, d] (lhsT = xT chunk).
  3. Per 128-row query chunk c: scores psum = qT.T @ kT over tk in [0,(c+1)*128);
     additive causal mask on the diagonal block; row-max (negated) -> bias of
     a single Exp activation that also emits the row-sum (accum_out).
  4. PE-transpose the exp'd weights, AV matmul (lhsT = wT), scale by 1/rowsum,
     DMA out.
"""

import numpy as np

import concourse.bass as bass
import concourse.mybir as mybir
import concourse.tile as tile
from concourse.bass_utils import run_bass_kernel_spmd
from concourse.masks import make_causal_mask, make_identity

B, T, DM, D = 64, 512, 1024, 256
NCORES = 8
BPC = B // NCORES  # batches per core
P = 128
KO = DM // P  # 8 contraction subtiles for the projections
NCH = T // P  # 4 token chunks per sequence
DJ = D // P  # 2 head-dim chunks
SCALE = 1.0 / 16.0  # 256 ** -0.5
MASK_VAL = -1e30

F32 = mybir.dt.float32
F32R = mybir.dt.float32r


def r(ap):
    """View an fp32 AP as float32r for full-rate PE matmuls."""
    return ap.bitcast(F32R)


def emit_core_program(nc: bass.Bass, tc, io):
    x_d, wq_d, bq_d, wk_d, bk_d, wv_d, bv_d, out_d = io
    X = mybir.AxisListType.X

    consts = tc.enter_pool("consts", bufs=1)
    ident = consts.tile([P, P], F32, name="ident")
    make_identity(nc, ident)
    cmask = consts.tile([P, P], F32, name="cmask")
    make_causal_mask(nc, cmask, mask_val=MASK_VAL)

    wq_s = consts.tile([P, KO, D], F32, name="wq_s")
    wk_s = consts.tile([P, KO, D], F32, name="wk_s")
    wv_s = consts.tile([P, KO, D], F32, name="wv_s")
    nc.sync.dma_start(wq_s, wq_d.rearrange("(ko p) d -> p ko d", p=P))
    nc.sync.dma_start(wk_s, wk_d.rearrange("(ko p) d -> p ko d", p=P))
    nc.sync.dma_start(wv_s, wv_d.rearrange("(ko p) d -> p ko d", p=P))
    bq_s = consts.tile([P, DJ], F32, name="bq_s")
    bk_s = consts.tile([P, DJ], F32, name="bk_s")
    nc.sync.dma_start(bq_s, bq_d.rearrange("(j p) -> p j", p=P))
    nc.sync.dma_start(bk_s, bk_d.rearrange("(j p) -> p j", p=P))
    bv_s = consts.tile([P, D], F32, name="bv_s")
    nc.sync.dma_start(bv_s, bv_d[None, :].to_broadcast((P, D)))

    x_pool = tc.enter_pool("x", bufs=2)
    xt_pool = tc.enter_pool("xt", bufs=2)
    qkv_pool = tc.enter_pool("qkv", bufs=2)
    w_pool = tc.enter_pool("w", bufs=2)
    wt_pool = tc.enter_pool("wt", bufs=2)
    o_pool = tc.enter_pool("o", bufs=2)
    stat_pool = tc.enter_pool("stat", bufs=8)
    ps_tr = tc.enter_pool("ps_tr", bufs=2, space="PSUM")
    ps_mm = tc.enter_pool("ps_mm", bufs=3, space="PSUM")
    ps_av = tc.enter_pool("ps_av", bufs=2, space="PSUM")

    for b in range(BPC):
        # ---- load + transpose x[b] ----
        x_sb = x_pool.tile([P, NCH, DM], F32, name="x_sb", tag="x_sb")
        nc.sync.dma_start(x_sb, x_d[b].rearrange("(c p) m -> p c m", p=P))
        xt = xt_pool.tile([P, KO, T], F32, name="xt", tag="xt")
        for c in range(NCH):
            for ko in range(KO):
                pt = ps_tr.tile([P, P], F32, name="pt", tag="pt")
                nc.tensor.transpose(pt, x_sb[:, c, ko * P:(ko + 1) * P], ident)
                nc.vector.tensor_copy(xt[:, ko, c * P:(c + 1) * P], pt)

        # ---- projections: qT, kT [128, DJ, T] (head dim on partitions) ----
        qt = qkv_pool.tile([P, DJ, T], F32, name="qt", tag="qt")
        kt = qkv_pool.tile([P, DJ, T], F32, name="kt", tag="kt")
        for w_s, b_s, dst, prescale in ((wq_s, bq_s, qt, True), (wk_s, bk_s, kt, False)):
            for j in range(DJ):
                pm = ps_mm.tile([P, T], F32, name="pm", tag="pm")
                for ko in range(KO):
                    nc.tensor.matmul(
                        pm,
                        r(w_s[:, ko, j * P:(j + 1) * P]),
                        r(xt[:, ko, :]),
                        start=(ko == 0),
                        stop=(ko == KO - 1),
                    )
                if prescale:
                    # (x@Wq + bq) * (1/sqrt(d)) folded into the copy-back
                    nc.vector.tensor_scalar(
                        dst[:, j, :], pm, b_s[:, j:j + 1], SCALE,
                        op0=mybir.AluOpType.add, op1=mybir.AluOpType.mult,
                    )
                else:
                    nc.vector.tensor_scalar(
                        dst[:, j, :], pm, b_s[:, j:j + 1], None,
                        op0=mybir.AluOpType.add,
                    )

        # ---- v in natural layout [tk-chunk on partitions, d free] ----
        v_sb = qkv_pool.tile([P, NCH, D], F32, name="v_sb", tag="v_sb")
        for c in range(NCH):
            pv = ps_av.tile([P, D], F32, name="pv", tag="pav")
            for ko in range(KO):
                nc.tensor.matmul(
                    pv,
                    r(xt[:, ko, c * P:(c + 1) * P]),
                    r(wv_s[:, ko, :]),
                    start=(ko == 0),
                    stop=(ko == KO - 1),
                )
            nc.vector.tensor_add(v_sb[:, c, :], pv, bv_s)

        # ---- attention, one 128-row query chunk at a time ----
        o_sb = o_pool.tile([P, NCH, D], F32, name="o_sb", tag="o_sb")
        for c in range(NCH):
            L = (c + 1) * P  # causal: keys [0, L)
            ps = ps_mm.tile([P, T], F32, name="ps", tag="pm")
            for j in range(DJ):
                nc.tensor.matmul(
                    ps[:, :L],
                    r(qt[:, j, c * P:(c + 1) * P]),
                    r(kt[:, j, :L]),
                    start=(j == 0),
                    stop=(j == DJ - 1),
                )
            # additive causal mask on the diagonal block
            nc.vector.tensor_add(ps[:, c * P:L], ps[:, c * P:L], cmask)
            nm = stat_pool.tile([P, 1], F32, name="nm", tag="nm")
            nc.vector.reduce_max(nm, ps[:, :L], axis=X, negate=True)
            w_sb = w_pool.tile([P, T], F32, name="w_sb", tag="w_sb")
            l_sb = stat_pool.tile([P, 1], F32, name="l_sb", tag="l_sb")
            nc.scalar.activation(
                w_sb[:, :L], ps[:, :L], mybir.ActivationFunctionType.Exp,
                bias=nm, scale=1.0, accum_out=l_sb,
            )
            linv = stat_pool.tile([P, 1], F32, name="linv", tag="linv")
            nc.vector.reciprocal(linv, l_sb)

            wt = wt_pool.tile([P, NCH, P], F32, name="wt", tag="wt")
            for s in range(c + 1):
                pt2 = ps_tr.tile([P, P], F32, name="pt2", tag="pt")
                nc.tensor.transpose(pt2, w_sb[:, s * P:(s + 1) * P], ident)
                nc.vector.tensor_copy(wt[:, s, :], pt2)

            po = ps_av.tile([P, D], F32, name="po", tag="pav")
            for s in range(c + 1):
                nc.tensor.matmul(
                    po, r(wt[:, s, :]), r(v_sb[:, s, :]),
                    start=(s == 0), stop=(s == c),
                )
            nc.vector.tensor_scalar(
                o_sb[:, c, :], po, linv, None, op0=mybir.AluOpType.mult,
            )

        nc.sync.dma_start(out_d[b].rearrange("(c p) d -> p c d", p=P), o_sb)


class _PoolCtx:
    """Small helper so emit_core_program can open pools without an ExitStack
    at every call site."""

    def __init__(self, tc):
        self.tc = tc
        self._stack = []

    def enter_pool(self, name, bufs, space="SBUF"):
        cm = self.tc.tile_pool(name=name, bufs=bufs, space=space)
        pool = cm.__enter__()
        self._stack.append(cm)
        return pool

    def close(self):
        for cm in reversed(self._stack):
            cm.__exit__(None, None, None)


def build_program():
    """Build the single-core Bass program (same program runs on all 8 cores)."""
    nc = bass.Bass("TRN2", target_bir_lowering=False, debug=False)
    x_d = nc.dram_tensor("x", [BPC, T, DM], F32, kind="ExternalInput").ap()
    wq_d = nc.dram_tensor("wq", [DM, D], F32, kind="ExternalInput").ap()
    bq_d = nc.dram_tensor("bq", [D], F32, kind="ExternalInput").ap()
    wk_d = nc.dram_tensor("wk", [DM, D], F32, kind="ExternalInput").ap()
    bk_d = nc.dram_tensor("bk", [D], F32, kind="ExternalInput").ap()
    wv_d = nc.dram_tensor("wv", [DM, D], F32, kind="ExternalInput").ap()
    bv_d = nc.dram_tensor("bv", [D], F32, kind="ExternalInput").ap()
    out_d = nc.dram_tensor("out", [BPC, T, D], F32, kind="ExternalOutput").ap()

    with tile.TileContext(nc) as tc:
        helper = _PoolCtx(tc)
        tc.enter_pool = helper.enter_pool
        try:
            emit_core_program(
                nc, tc, (x_d, wq_d, bq_d, wk_d, bk_d, wv_d, bv_d, out_d)
            )
        finally:
            helper.close()
    return nc


_NC_CACHE = None


def _get_program():
    global _NC_CACHE
    if _NC_CACHE is None:
        _NC_CACHE = build_program()
    return _NC_CACHE


def make_in_maps(inputs):
    x = np.ascontiguousarray(np.asarray(inputs["x"], dtype=np.float32))
    shared = {
        "wq": np.ascontiguousarray(np.asarray(inputs["Wq"], np.float32)),
        "bq": np.ascontiguousarray(np.asarray(inputs["bq"], np.float32)),
        "wk": np.ascontiguousarray(np.asarray(inputs["Wk"], np.float32)),
        "bk": np.ascontiguousarray(np.asarray(inputs["bk"], np.float32)),
        "wv": np.ascontiguousarray(np.asarray(inputs["Wv"], np.float32)),
        "bv": np.ascontiguousarray(np.asarray(inputs["bv"], np.float32)),
    }
    return [
        {"x": x[i * BPC:(i + 1) * BPC], **shared} for i in range(NCORES)
    ]


def kernel(**inputs) -> np.ndarray:
    nc = _get_program()
    in_maps = make_in_maps(inputs)
    res = run_bass_kernel_spmd(nc, in_maps, core_ids=list(range(NCORES)))
    return np.concatenate([m["out"] for m in res.results], axis=0)


# revision 6
# speedup vs baseline: 2.5819x; 2.5819x over previous
"""Causal attention kernel for Trainium2 (Bass/Tile), data-parallel over batch.

Problem (hardcoded): x[64,512,1024] f32, Wq/Wk/Wv[1024,256], bq/bk/bv[256].
  q = x@Wq+bq ; k = x@Wk+bk ; v = x@Wv+bv
  out = softmax(causal(q k^T / sqrt(256))) @ v           -> [64,512,256]

Sharding: 8 NeuronCores, 8 batches per core (pure data parallel, weights
replicated, no collectives). Each core runs the same program on its shard.

Per-core pipeline (per batch):
  1. DMA x[b] -> SBUF [128,4,1024]; PE-transpose 128x128 tiles -> xT [128,8,512]
  2. qT/kT = W.T @ x.T via fp32r matmuls (d on partitions), bias folded into
     the PSUM->SBUF copy; q pre-scaled by 1/sqrt(d).
     v computed in natural layout [tk,128-chunks, d] (lhsT = xT chunk).
  3. Per 128-row query chunk c: scores psum = qT.T @ kT over tk in [0,(c+1)*128);
     additive causal mask on the diagonal block; row-max (negated) -> bias of
     a single Exp activation that also emits the row-sum (accum_out).
  4. PE-transpose the exp'd weights, AV matmul (lhsT = wT), scale by 1/rowsum,
     DMA out.
"""

import numpy as np

import concourse.bass as bass
import concourse.mybir as mybir
import concourse.tile as tile
from concourse import bacc
from concourse.bass_utils import run_bass_kernel_spmd
from concourse.masks import make_causal_mask, make_identity

B, T, DM, D = 64, 512, 1024, 256
NCORES = 8
BPC = B // NCORES  # batches per core
P = 128
KO = DM // P  # 8 contraction subtiles for the projections
NCH = T // P  # 4 token chunks per sequence
DJ = D // P  # 2 head-dim chunks
SCALE = 1.0 / 16.0  # 256 ** -0.5
MASK_VAL = -1e30

F32 = mybir.dt.float32
F32R = mybir.dt.float32r


def emit_core_program(ctx, nc: bass.Bass, tc, io, reps=1):
    x_d, wq_d, bq_d, wk_d, bk_d, wv_d, bv_d, out_d = io
    X = mybir.AxisListType.X

    def enter_pool(name, bufs, space="SBUF"):
        return ctx.enter_context(tc.tile_pool(name=name, bufs=bufs, space=space))

    consts = enter_pool("consts", bufs=1)
    ident = consts.tile([P, P], F32, name="ident")
    make_identity(nc, ident)
    cmask = consts.tile([P, P], F32, name="cmask")
    make_causal_mask(nc, cmask, mask_val=MASK_VAL)

    wq_s = consts.tile([P, KO, D], F32R, name="wq_s")
    wk_s = consts.tile([P, KO, D], F32R, name="wk_s")
    wv_s = consts.tile([P, KO, D], F32R, name="wv_s")
    nc.sync.dma_start(wq_s, wq_d.rearrange("(ko p) d -> p ko d", p=P).bitcast(F32R))
    nc.sync.dma_start(wk_s, wk_d.rearrange("(ko p) d -> p ko d", p=P).bitcast(F32R))
    nc.sync.dma_start(wv_s, wv_d.rearrange("(ko p) d -> p ko d", p=P).bitcast(F32R))
    bq_s = consts.tile([P, DJ], F32, name="bq_s")
    bk_s = consts.tile([P, DJ], F32, name="bk_s")
    nc.sync.dma_start(bq_s, bq_d.rearrange("(j p) -> p j", p=P))
    nc.sync.dma_start(bk_s, bk_d.rearrange("(j p) -> p j", p=P))
    bv_s = consts.tile([P, D], F32, name="bv_s")
    nc.sync.dma_start(bv_s, bv_d[None, :].to_broadcast((P, D)))

    x_pool = enter_pool("x", bufs=2)
    xt_pool = enter_pool("xt", bufs=2)
    qkv_pool = enter_pool("qkv", bufs=2)
    w_pool = enter_pool("w", bufs=2)
    wt_pool = enter_pool("wt", bufs=2)
    o_pool = enter_pool("o", bufs=2)
    stat_pool = enter_pool("stat", bufs=8)
    ps_tr = enter_pool("ps_tr", bufs=2, space="PSUM")
    ps_mm = enter_pool("ps_mm", bufs=3, space="PSUM")
    ps_av = enter_pool("ps_av", bufs=2, space="PSUM")

    if reps > 1:
        ctx.enter_context(tc.For_i(0, reps, 1))

    for b in range(BPC):
        # ---- load + transpose x[b] ----
        x_sb = x_pool.tile([P, NCH, DM], F32, name="x_sb", tag="x_sb")
        nc.sync.dma_start(x_sb, x_d[b].rearrange("(c p) m -> p c m", p=P))
        xt = xt_pool.tile([P, KO, T], F32R, name="xt", tag="xt")
        for c in range(NCH):
            for ko in range(KO):
                pt = ps_tr.tile([P, P], F32, name="pt", tag="pt")
                nc.tensor.transpose(pt, x_sb[:, c, ko * P:(ko + 1) * P], ident)
                nc.vector.tensor_copy(xt[:, ko, c * P:(c + 1) * P], pt)

        # ---- projections: qT, kT [128, DJ, T] (head dim on partitions) ----
        qt = qkv_pool.tile([P, DJ, T], F32R, name="qt", tag="qt")
        kt = qkv_pool.tile([P, DJ, T], F32R, name="kt", tag="kt")
        for w_s, b_s, dst, prescale in ((wq_s, bq_s, qt, True), (wk_s, bk_s, kt, False)):
            for j in range(DJ):
                pm = ps_mm.tile([P, T], F32, name="pm", tag="pm")
                for ko in range(KO):
                    nc.tensor.matmul(
                        pm,
                        w_s[:, ko, j * P:(j + 1) * P],
                        xt[:, ko, :],
                        start=(ko == 0),
                        stop=(ko == KO - 1),
                    )
                if prescale:
                    # (x@Wq + bq) * (1/sqrt(d)) folded into the copy-back
                    nc.vector.tensor_scalar(
                        dst[:, j, :], pm, b_s[:, j:j + 1], SCALE,
                        op0=mybir.AluOpType.add, op1=mybir.AluOpType.mult,
                    )
                else:
                    nc.vector.tensor_scalar(
                        dst[:, j, :], pm, b_s[:, j:j + 1], None,
                        op0=mybir.AluOpType.add,
                    )

        # ---- v in natural layout [tk-chunk on partitions, d free] ----
        v_sb = qkv_pool.tile([P, NCH, D], F32R, name="v_sb", tag="v_sb")
        for c in range(NCH):
            pv = ps_av.tile([P, D], F32, name="pv", tag="pav")
            for ko in range(KO):
                nc.tensor.matmul(
                    pv,
                    xt[:, ko, c * P:(c + 1) * P],
                    wv_s[:, ko, :],
                    start=(ko == 0),
                    stop=(ko == KO - 1),
                )
            nc.vector.tensor_add(v_sb[:, c, :], pv, bv_s)

        # ---- attention, one 128-row query chunk at a time ----
        o_sb = o_pool.tile([P, NCH, D], F32, name="o_sb", tag="o_sb")
        for c in range(NCH):
            L = (c + 1) * P  # causal: keys [0, L)
            ps = ps_mm.tile([P, T], F32, name="ps", tag="pm")
            for j in range(DJ):
                nc.tensor.matmul(
                    ps[:, :L],
                    qt[:, j, c * P:(c + 1) * P],
                    kt[:, j, :L],
                    start=(j == 0),
                    stop=(j == DJ - 1),
                )
            # additive causal mask on the diagonal block
            nc.vector.tensor_add(ps[:, c * P:L], ps[:, c * P:L], cmask)
            nm = stat_pool.tile([P, 1], F32, name="nm", tag="nm")
            nc.vector.reduce_max(nm, ps[:, :L], axis=X, negate=True)
            w_sb = w_pool.tile([P, T], F32, name="w_sb", tag="w_sb")
            l_sb = stat_pool.tile([P, 1], F32, name="l_sb", tag="l_sb")
            nc.scalar.activation(
                w_sb[:, :L], ps[:, :L], mybir.ActivationFunctionType.Exp,
                bias=nm, scale=1.0, accum_out=l_sb,
            )
            linv = stat_pool.tile([P, 1], F32, name="linv", tag="linv")
            nc.vector.reciprocal(linv, l_sb)

            wt = wt_pool.tile([P, NCH, P], F32R, name="wt", tag="wt")
            for s in range(c + 1):
                pt2 = ps_tr.tile([P, P], F32, name="pt2", tag="pt")
                nc.tensor.transpose(pt2, w_sb[:, s * P:(s + 1) * P], ident)
                nc.vector.tensor_copy(wt[:, s, :], pt2)

            po = ps_av.tile([P, D], F32, name="po", tag="pav")
            for s in range(c + 1):
                nc.tensor.matmul(
                    po, wt[:, s, :], v_sb[:, s, :],
                    start=(s == 0), stop=(s == c),
                )
            nc.vector.tensor_scalar(
                o_sb[:, c, :], po, linv, None, op0=mybir.AluOpType.mult,
            )

        nc.sync.dma_start(out_d[b].rearrange("(c p) d -> p c d", p=P), o_sb)


def build_program(reps=1):
    """Build the single-core Bass program (same program runs on all 8 cores).

    reps > 1 wraps the whole body in a hardware loop (same work each
    iteration) -- used only for device-time measurement."""
    nc = bacc.Bacc("TRN2", target_bir_lowering=False, debug=False)
    x_d = nc.dram_tensor("x", [BPC, T, DM], F32, kind="ExternalInput").ap()
    wq_d = nc.dram_tensor("wq", [DM, D], F32, kind="ExternalInput").ap()
    bq_d = nc.dram_tensor("bq", [D], F32, kind="ExternalInput").ap()
    wk_d = nc.dram_tensor("wk", [DM, D], F32, kind="ExternalInput").ap()
    bk_d = nc.dram_tensor("bk", [D], F32, kind="ExternalInput").ap()
    wv_d = nc.dram_tensor("wv", [DM, D], F32, kind="ExternalInput").ap()
    bv_d = nc.dram_tensor("bv", [D], F32, kind="ExternalInput").ap()
    out_d = nc.dram_tensor("out", [BPC, T, D], F32, kind="ExternalOutput").ap()

    from contextlib import ExitStack

    with tile.TileContext(nc) as tc, ExitStack() as ctx:
        emit_core_program(
            ctx, nc, tc, (x_d, wq_d, bq_d, wk_d, bk_d, wv_d, bv_d, out_d),
            reps=reps,
        )
    nc.compile()
    return nc


_NC_CACHE = None


def _get_program():
    global _NC_CACHE
    if _NC_CACHE is None:
        _NC_CACHE = build_program()
    return _NC_CACHE


def make_in_maps(inputs):
    x = np.ascontiguousarray(np.asarray(inputs["x"], dtype=np.float32))
    shared = {
        "wq": np.ascontiguousarray(np.asarray(inputs["Wq"], np.float32)),
        "bq": np.ascontiguousarray(np.asarray(inputs["bq"], np.float32)),
        "wk": np.ascontiguousarray(np.asarray(inputs["Wk"], np.float32)),
        "bk": np.ascontiguousarray(np.asarray(inputs["bk"], np.float32)),
        "wv": np.ascontiguousarray(np.asarray(inputs["Wv"], np.float32)),
        "bv": np.ascontiguousarray(np.asarray(inputs["bv"], np.float32)),
    }
    return [
        {"x": x[i * BPC:(i + 1) * BPC], **shared} for i in range(NCORES)
    ]


def kernel(**inputs) -> np.ndarray:
    nc = _get_program()
    in_maps = make_in_maps(inputs)
    res = run_bass_kernel_spmd(nc, in_maps, core_ids=list(range(NCORES)))
    return np.concatenate([m["out"] for m in res.results], axis=0)
